# revision 3
# baseline (speedup 1.0000x reference)
"""Trainium2 Bass kernel: causal multi-head group attention (GQA) with RoPE.

Full-input contract: kernel(**inputs) takes the unsharded inputs and returns
the full output. Internally shards across 8 NeuronCores:
  core c -> (batch b = c // 4, head-group g = c % 4)
Each core computes 4 q heads + their single kv group end-to-end (QKV proj,
RoPE, causal flash-style attention, row-parallel out-proj partial). Host sums
the 4 per-group partials per batch and adds the output bias (row-parallel Wo
unshard-reduce).

All matmuls run as float32r (full-rate fp32, TF32-like mantissa rounding).
"""

import os
import sys
from contextlib import ExitStack
from math import sqrt

for _p in ("/opt/trn_rl_repo", "/root/.axon_site/_ro/trn_rl_repo"):
    if os.path.isdir(_p) and _p not in sys.path:
        sys.path.insert(0, _p)

import numpy as np
import concourse.bacc as bacc
import concourse.tile as tile
import concourse.mybir as mybir
from concourse.bass_utils import run_bass_kernel_spmd

F32 = mybir.dt.float32
F32R = mybir.dt.float32r
EXP = mybir.ActivationFunctionType.Exp

N_CORES = 8
TP = 4            # head-group parallel degree (within one batch element)
BATCH = 2
D = 128           # head dim
NHL = 4           # q heads per core
ROPE_BASE = 10000.0

# Full-problem config
S_FULL = 2048     # context length
E_FULL = 2048     # model dim


def build_program(S, E, QC=512, PW=512, n_cores=N_CORES):
    """Emit the per-core SPMD program. QC: q-chunk width (attention moving dim),
    PW: phase-1 pass width over the sequence."""
    EC = E // 128     # contraction chunks over model dim
    NKI = S // 128    # k tiles
    NQJ = S // QC     # q chunks
    NSP = S // PW     # phase-1 s-passes
    B = QC - 128      # mask-table base offset
    MW = 2 * QC - 128 # mask-table width

    nc = bacc.Bacc("TRN2", target_bir_lowering=False, debug=False,
                   num_devices=n_cores)

    xT = nc.dram_tensor("xT", [E, S], F32R, kind="ExternalInput").ap()
    Wq = nc.dram_tensor("Wq", [E, NHL * D], F32R, kind="ExternalInput").ap()
    Wk = nc.dram_tensor("Wk", [E, D], F32R, kind="ExternalInput").ap()
    Wv = nc.dram_tensor("Wv", [E, D], F32R, kind="ExternalInput").ap()
    Wo = nc.dram_tensor("Wo", [NHL * D, E], F32R, kind="ExternalInput").ap()
    sinT = nc.dram_tensor("sinT", [D, S], F32, kind="ExternalInput").ap()
    sinTs = nc.dram_tensor("sinTs", [D, S], F32, kind="ExternalInput").ap()
    mwide = nc.dram_tensor("mwide", [128, MW], F32R, kind="ExternalInput").ap()
    ones_col = nc.dram_tensor("ones_col", [128, 1], F32R, kind="ExternalInput").ap()
    ones_row = nc.dram_tensor("ones_row", [1, 128], F32R, kind="ExternalInput").ap()
    ident = nc.dram_tensor("ident", [128, 128], F32R, kind="ExternalInput").ap()
    out = nc.dram_tensor("out", [S, E], F32, kind="ExternalOutput").ap()

    with tile.TileContext(nc) as tc, ExitStack() as top:
        pers = top.enter_context(tc.tile_pool(name="pers", bufs=1))
        qT_sb = [pers.tile([128, S], F32R, tag=f"qT{h}", name=f"qT{h}")
                 for h in range(NHL)]
        kT_sb = pers.tile([128, S], F32R, name="kT_sb")
        v_sb = [pers.tile([128, D], F32R, tag=f"v{i}", name=f"v{i}")
                for i in range(NKI)]
        yT_sb = [pers.tile([128, S], F32R, tag=f"yT{h}", name=f"yT{h}")
                 for h in range(NHL)]

        # ---------------- Phase 1: QKV projections + RoPE -----------------
        with ExitStack() as ph1:
            wpool = ph1.enter_context(tc.tile_pool(name="wts", bufs=1))
            wq = [wpool.tile([128, NHL * D], F32R, tag=f"wq{e}", name=f"wq{e}")
                  for e in range(EC)]
            wk = [wpool.tile([128, D], F32R, tag=f"wk{e}", name=f"wk{e}")
                  for e in range(EC)]
            wv = [wpool.tile([128, D], F32R, tag=f"wv{e}", name=f"wv{e}")
                  for e in range(EC)]
            sin_sb = wpool.tile([128, S], F32, tag="sin", name="sin_sb")
            sins_sb = wpool.tile([128, S], F32, tag="sins", name="sins_sb")
            ident_sb = wpool.tile([128, 128], F32R, tag="ident", name="ident_sb")
            for e in range(EC):
                nc.sync.dma_start(wq[e][:], Wq[128 * e:128 * (e + 1), :])
                nc.sync.dma_start(wk[e][:], Wk[128 * e:128 * (e + 1), :])
                nc.sync.dma_start(wv[e][:], Wv[128 * e:128 * (e + 1), :])
            nc.sync.dma_start(sin_sb[:], sinT[:])
            nc.sync.dma_start(sins_sb[:], sinTs[:])
            nc.sync.dma_start(ident_sb[:], ident[:])

            xpool = ph1.enter_context(tc.tile_pool(name="xt", bufs=1))
            psA = ph1.enter_context(tc.tile_pool(name="psA", bufs=2, space="PSUM"))
            stg = ph1.enter_context(tc.tile_pool(name="stg", bufs=2))

            for sp in range(NSP):
                xts = [xpool.tile([128, PW], F32R, tag=f"x{e}", name=f"x{e}_{sp}")
                       for e in range(EC)]
                for e in range(EC):
                    nc.sync.dma_start(
                        xts[e][:], xT[128 * e:128 * (e + 1), PW * sp:PW * (sp + 1)])
                for sub in range(PW // 512):
                    s0 = PW * sp + 512 * sub
                    # q heads (transposed layout [d, s])
                    for h in range(NHL):
                        ps = psA.tile([128, 512], F32, tag="proj", name=f"psq{sp}_{sub}_{h}")
                        for e in range(EC):
                            nc.tensor.matmul(
                                ps[:], wq[e][:, D * h:D * (h + 1)],
                                xts[e][:, 512 * sub:512 * (sub + 1)],
                                start=(e == 0), stop=(e == EC - 1))
                        nc.vector.tensor_copy(qT_sb[h][:, s0:s0 + 512], ps[:])
                    # k (transposed layout [d, s])
                    ps = psA.tile([128, 512], F32, tag="proj", name=f"psk{sp}_{sub}")
                    for e in range(EC):
                        nc.tensor.matmul(
                            ps[:], wk[e][:], xts[e][:, 512 * sub:512 * (sub + 1)],
                            start=(e == 0), stop=(e == EC - 1))
                    nc.vector.tensor_copy(kT_sb[:, s0:s0 + 512], ps[:])
                    # v: compute vT then PE-transpose into [s, d] tiles
                    ps = psA.tile([128, 512], F32, tag="proj", name=f"psv{sp}_{sub}")
                    for e in range(EC):
                        nc.tensor.matmul(
                            ps[:], wv[e][:], xts[e][:, 512 * sub:512 * (sub + 1)],
                            start=(e == 0), stop=(e == EC - 1))
                    vstage = stg.tile([128, 512], F32R, tag="vstage", name=f"vst{sp}_{sub}")
                    nc.vector.tensor_copy(vstage[:], ps[:])
                    for j in range(4):
                        vt_ps = psA.tile([128, 128], F32R, tag="vtr", name=f"vtr{sp}_{sub}_{j}")
                        nc.tensor.transpose(vt_ps[:], vstage[:, 128 * j:128 * (j + 1)],
                                            ident_sb[:])
                        nc.vector.tensor_copy(v_sb[s0 // 128 + j][:], vt_ps[:])

            # RoPE (buggy-faithful: q' = q*sin + rot_half(q)*sin)
            for t in qT_sb + [kT_sb]:
                tmp = stg.tile([128, S], F32R, tag="ropetmp", name="ropetmp")
                nc.sync.dma_start(tmp[0:64, :], t[64:128, :])
                nc.sync.dma_start(tmp[64:128, :], t[0:64, :])
                nc.vector.tensor_mul(tmp[:], tmp[:], sins_sb[:])
                nc.vector.tensor_mul(t[:], t[:], sin_sb[:])
                nc.vector.tensor_add(t[:], t[:], tmp[:])

        # ---------------- Phase 2: causal attention -----------------------
        with ExitStack() as ph2:
            mpool = ph2.enter_context(tc.tile_pool(name="mw", bufs=1))
            mw_sb = mpool.tile([128, MW], F32R, tag="mw", name="mw_sb")
            onesc = mpool.tile([128, 1], F32R, tag="onesc", name="onesc")
            onesr = mpool.tile([1, 128], F32R, tag="onesr", name="onesr")
            nc.sync.dma_start(mw_sb[:], mwide[:])
            nc.sync.dma_start(onesc[:], ones_col[:])
            nc.sync.dma_start(onesr[:], ones_row[:])

            ptpool = ph2.enter_context(tc.tile_pool(name="ptp", bufs=3))
            pspool = ph2.enter_context(tc.tile_pool(name="ps2", bufs=3, space="PSUM"))
            ypool = ph2.enter_context(tc.tile_pool(name="yps", bufs=2, space="PSUM"))

            scale = 1.0 / sqrt(D)
            for h in range(NHL):
                for qj in range(NQJ):
                    yps = ypool.tile([128, QC], F32, tag="yps", name=f"yps{h}_{qj}")
                    rs = ptpool.tile([128, QC], F32R, tag="rs", bufs=2, name=f"rs{h}_{qj}")
                    nki_hi = (qj + 1) * QC // 128
                    for ki in range(nki_hi):
                        st = pspool.tile([128, QC], F32, tag="st", name=f"st{h}_{qj}_{ki}")
                        nc.tensor.matmul(
                            st[:], kT_sb[:, 128 * ki:128 * (ki + 1)],
                            qT_sb[h][:, QC * qj:QC * (qj + 1)],
                            start=True, stop=True)
                        pt = ptpool.tile([128, QC], F32R, tag="pt", name=f"pt{h}_{qj}_{ki}")
                        nc.scalar.activation(pt[:], st[:], EXP, scale=scale)
                        off = 128 * ki - QC * qj
                        if off >= 0:
                            nc.vector.tensor_mul(pt[:], pt[:],
                                                 mw_sb[:, B - off:B - off + QC])
                        if ki == 0:
                            nc.vector.tensor_copy(rs[:], pt[:])
                        else:
                            nc.vector.tensor_add(rs[:], rs[:], pt[:])
                        nc.tensor.matmul(yps[:], v_sb[ki][:], pt[:],
                                         start=(ki == 0), stop=(ki == nki_hi - 1))
                    rsum = pspool.tile([1, QC], F32, tag="rsum", bufs=1, name=f"rsum{h}_{qj}")
                    nc.tensor.matmul(rsum[:], onesc[:], rs[:], start=True, stop=True)
                    rinv = ptpool.tile([1, QC], F32R, tag="rinv", bufs=2, name=f"rinv{h}_{qj}")
                    with nc.allow_low_precision(reason="f32r softmax denominator"):
                        nc.vector.reciprocal(rinv[:], rsum[:])
                    rb_ps = pspool.tile([128, QC], F32, tag="rb", bufs=1, name=f"rb{h}_{qj}")
                    nc.tensor.matmul(rb_ps[:], onesr[:], rinv[:], start=True, stop=True)
                    rb = ptpool.tile([128, QC], F32, tag="rbs", bufs=2, name=f"rbs{h}_{qj}")
                    nc.vector.tensor_copy(rb[:], rb_ps[:])
                    nc.vector.tensor_mul(yT_sb[h][:, QC * qj:QC * (qj + 1)],
                                         yps[:], rb[:])

        # ---------------- Phase 3: out-proj partial -----------------------
        with ExitStack() as ph3:
            wopool = ph3.enter_context(tc.tile_pool(name="wo", bufs=1))
            wo_sb = [wopool.tile([128, E], F32R, tag=f"wo{h}", name=f"wo{h}")
                     for h in range(NHL)]
            for h in range(NHL):
                nc.sync.dma_start(wo_sb[h][:], Wo[128 * h:128 * (h + 1), :])
            ps3 = ph3.enter_context(tc.tile_pool(name="ps3", bufs=4, space="PSUM"))
            opool = ph3.enter_context(tc.tile_pool(name="osb", bufs=2))
            for si in range(S // 128):
                osb = opool.tile([128, E], F32, tag="osb", name=f"osb{si}")
                for nj in range(E // 512):
                    ops = ps3.tile([128, 512], F32, tag="ops", name=f"ops{si}_{nj}")
                    for h in range(NHL):
                        nc.tensor.matmul(
                            ops[:], yT_sb[h][:, 128 * si:128 * (si + 1)],
                            wo_sb[h][:, 512 * nj:512 * (nj + 1)],
                            start=(h == 0), stop=(h == NHL - 1))
                    nc.vector.tensor_copy(osb[:, 512 * nj:512 * (nj + 1)], ops[:])
                nc.sync.dma_start(out[128 * si:128 * (si + 1), :], osb[:])

    nc.compile()
    return nc


def make_consts(S, QC=512):
    """Host-precomputed constant tensors (rope table, causal mask, ones, identity)."""
    rope_dim = D // 2
    j = np.arange(rope_dim, dtype=np.float64)
    thetas = 1.0 / ROPE_BASE ** (2.0 * j / rope_dim)
    positions = np.arange(S, dtype=np.float64)
    angles = positions[:, None] * thetas[None, :]
    sin = np.sin(np.concatenate([angles, angles], axis=1)).astype(np.float32)  # [S, D]
    sinT = np.ascontiguousarray(sin.T)                                          # [D, S]
    sgn = np.where(np.arange(D) < rope_dim, -1.0, 1.0).astype(np.float32)
    sinTs = np.ascontiguousarray(sinT * sgn[:, None])

    B = QC - 128
    MW = 2 * QC - 128
    k_idx = np.arange(128)[:, None]
    c_idx = np.arange(MW)[None, :]
    mw = (k_idx <= (c_idx - B)).astype(np.float32)

    return {
        "sinT": sinT,
        "sinTs": sinTs,
        "mwide": np.ascontiguousarray(mw),
        "ones_col": np.ones((128, 1), np.float32),
        "ones_row": np.ones((1, 128), np.float32),
        "ident": np.eye(128, dtype=np.float32),
    }


def make_in_maps(x, Wq, Wk, Wv, Wo, S, E, QC=512):
    """Shard full inputs into the 8 per-core input maps."""
    consts = make_consts(S, QC)
    in_maps = []
    for c in range(N_CORES):
        b, g = c // TP, c % TP
        m = dict(consts)
        m["xT"] = np.ascontiguousarray(x[b].T)
        m["Wq"] = np.ascontiguousarray(Wq[:, NHL * D * g:NHL * D * (g + 1)])
        m["Wk"] = np.ascontiguousarray(Wk[:, D * g:D * (g + 1)])
        m["Wv"] = np.ascontiguousarray(Wv[:, D * g:D * (g + 1)])
        m["Wo"] = np.ascontiguousarray(Wo[NHL * D * g:NHL * D * (g + 1), :])
        in_maps.append(m)
    return in_maps


_CACHE = {}


def _compiled_full():
    if "nc" not in _CACHE:
        _CACHE["nc"] = build_program(S_FULL, E_FULL)
    return _CACHE["nc"]


def kernel(x, Wq, Wk, Wv, Wo, bo, _trace=False):
    nc = _compiled_full()
    in_maps = make_in_maps(x, Wq, Wk, Wv, Wo, S_FULL, E_FULL)
    res = run_bass_kernel_spmd(nc, in_maps, list(range(N_CORES)), trace=_trace)
    out = np.zeros((BATCH, S_FULL, E_FULL), np.float32)
    for c in range(N_CORES):
        out[c // TP] += res.results[c]["out"]
    out += bo[None, None, :].astype(np.float32)
    if _trace:
        _CACHE["last_exec_time_ns"] = res.exec_time_ns
    return out
